# revision 3
# baseline (speedup 1.0000x reference)
"""Trainium2 Bass kernel for nn_DivEncoder (grouped MLP + ELU + L2 norm).

Math (per batch row n, group d):
  zeta = einsum('duv,dv->du', W1, xg) + b1;  y_d = b2_d + sum_u W2[d,u]*elu(zeta)
  elu(z) = z - m + e - 1,  m = min(z,0), e = exp(m)
  y = c0 + wlin.x + W2 e - W2 m;  c0 = b2 + sum_u W2 b1 - sum_u W2;
  wlin = sum_u W2[d,u] W1[d,u,:]
  out = y / max(||y||, eps)

Sharding: batch rows across 8 cores (512 rows each); weights replicated.

Key structure (HW-tuned; measured 289us vs 530us for the per-chunk-DMA
predecessor):
  - Weights preloaded with ONE batched DMA per class on the scalar HWDGE
    ring (per-chunk weight DMAs serialize the SP sequencer for ~190us).
  - All x loads + xbar transposes issue on the SP HWDGE ring at 2-chunk
    (superchunk) granularity; Pool (gpsimd) does the fp32->fp16 cast.
  - m-strips (PSUM->SBUF min+bias) on DVE; exp batched per superchunk as
    one [128,4096] ACT instruction.
  - The LAST superchunk of each PSUM bank group (s%8==7) runs fully on
    ACT (q=relu(-zeta), e=exp(-q), host flips b1/w2m signs) so DVE is
    free for the bank evac. This alignment is a sharp optimum: rotation
    at 0, 1/6, or 1/4 all measure ~440us vs 289us here.
"""
import sys
sys.path.insert(0, "/opt/trn_rl_repo")

import numpy as np
import ml_dtypes

import concourse.bass as bass
import concourse.bacc as bacc
import concourse.mybir as mybir
import concourse.tile as tile
from concourse import bass_utils

F32 = mybir.dt.float32
F16 = mybir.dt.float16
BF16 = mybir.dt.bfloat16
AL = mybir.AluOpType
AF = mybir.ActivationFunctionType

ABLATE_EM = ABLATE_ME = ABLATE_L1 = False

N, H, D, U, V = 4096, 8192, 512, 64, 16
NCORE = 8
R = N // NCORE          # 512 batch rows per core
CH = H // 128           # 64 chunks
SC = CH // 2            # 32 superchunks (2 chunks each)
BG = 4                  # bank groups (16 chunks each)
EPS = 1e-12

_cache = {}


def _act_chain(c):
    """Chunks fully handled on ACT (q=relu(-zeta), e=exp(-q))."""
    return (c // 2) % 8 == 7


def _build(loop_reps=1):
    nc = bacc.Bacc("TRN2", target_bir_lowering=False, debug=False,
                   enable_asserts=False, num_devices=NCORE)
    ap = {}
    ap["x"] = nc.dram_tensor("x", [R, H], F32, kind="ExternalInput").ap()
    ap["w1f"] = nc.dram_tensor("w1f", [128, CH * 128], F16, kind="ExternalInput").ap()
    ap["wlh"] = nc.dram_tensor("wlh", [128, CH * 128], F16, kind="ExternalInput").ap()
    ap["wlr"] = nc.dram_tensor("wlr", [128, CH * 128], F16, kind="ExternalInput").ap()
    ap["w2e"] = nc.dram_tensor("w2e", [128, CH * 128], F16, kind="ExternalInput").ap()
    ap["w2m"] = nc.dram_tensor("w2m", [128, CH * 128], F16, kind="ExternalInput").ap()
    ap["b1c"] = nc.dram_tensor("b1c", [128, CH * 4], F32, kind="ExternalInput").ap()
    ap["c0s"] = nc.dram_tensor("c0s", [128, BG], F32, kind="ExternalInput").ap()
    ap["ident"] = nc.dram_tensor("ident", [128, 128], F32, kind="ExternalInput").ap()
    y_out = nc.dram_tensor("y", [R, D], F32, kind="ExternalOutput").ap()

    with tile.TileContext(nc) as tc:
        _emit(nc, tc, ap, y_out, loop_reps)
    nc.compile()
    return nc


def _emit(nc, tc, ap, y_out, loop_reps=1):
    with (
        tc.tile_pool(name="wres", bufs=1) as wres,
        tc.tile_pool(name="xin", bufs=3) as xin,
        tc.tile_pool(name="xsp", bufs=3) as xsp,
        tc.tile_pool(name="xtr", bufs=3) as xtr,
        tc.tile_pool(name="me", bufs=3) as mepool,
        tc.tile_pool(name="yfm", bufs=1) as yfm,
        tc.tile_pool(name="zps", bufs=3, space="PSUM") as zps,
        tc.tile_pool(name="yps", bufs=2, space="PSUM") as yps,
        tc.tile_pool(name="sml", bufs=1) as sml,
    ):
        # ---- resident weights: one batched DMA per class, scalar ring,
        # ordered by first use (L1 -> bias -> wlin -> L2 -> consts)
        t_w1 = wres.tile([128, CH * 128], F16, tag="w1", name="t_w1")
        nc.scalar.dma_start(t_w1[:], ap["w1f"][:])
        t_b1 = wres.tile([128, CH * 4], F32, tag="b1", name="t_b1")
        nc.scalar.dma_start(t_b1[:], ap["b1c"][:])
        t_wlh = wres.tile([128, CH * 128], F16, tag="wlh", name="t_wlh")
        nc.scalar.dma_start(t_wlh[:], ap["wlh"][:])
        t_wlr = wres.tile([128, CH * 128], F16, tag="wlr", name="t_wlr")
        nc.scalar.dma_start(t_wlr[:], ap["wlr"][:])
        t_w2e = wres.tile([128, CH * 128], F16, tag="w2e", name="t_w2e")
        nc.scalar.dma_start(t_w2e[:], ap["w2e"][:])
        t_w2m = wres.tile([128, CH * 128], F16, tag="w2m", name="t_w2m")
        nc.scalar.dma_start(t_w2m[:], ap["w2m"][:])
        t_c0 = wres.tile([128, BG], F32, tag="c0", name="t_c0")
        nc.scalar.dma_start(t_c0[:], ap["c0s"][:])
        t_id = wres.tile([128, 128], F32, tag="ident", name="t_id")
        nc.scalar.dma_start(t_id[:], ap["ident"][:])

        x_ap = ap["x"]
        import contextlib
        loop_cm = tc.For_i(0, loop_reps, 1) if loop_reps > 1 else contextlib.nullcontext()
        with loop_cm:
            y_banks = {}
            t_yfm = [yfm.tile([128, 512], F32, tag=f"yfm{b}", name=f"yfm{b}")
                     for b in range(BG)]

            pend = [None]
            for s in range(SC + 1):
                if s < SC:
                    act_sc = _act_chain(2 * s)
                    # --- load 2 chunks of x: [128 rows, 2 c x 4 j x 128 f]
                    xt = xin.tile([128, 1024], F32, tag="xt", name=f"xt{s}")
                    for cc in range(2):
                        c = 2 * s + cc
                        nc.sync.dma_start(
                            xt[:, 512 * cc:512 * (cc + 1)].rearrange(
                                "p (j f) -> p j f", j=4),
                            x_ap[:, 128 * c:128 * (c + 1)].rearrange(
                                "(j p) f -> p j f", p=128))
                    # --- cast fp32 -> fp16 on Pool
                    xf = xsp.tile([128, 1024], F16, tag="xf", name=f"xf{s}")
                    nc.gpsimd.tensor_copy(xf[:], xt[:])
                    # --- one batched xbar transpose (8 128x128 blocks)
                    xfT = xtr.tile([128, 1024], F16, tag="xfT", name=f"xfT{s}")
                    nc.sync.dma_start_transpose(
                        xfT[:].rearrange("p (b n) -> p b n", b=8), xf[:])

                    m2 = mepool.tile([128, 4096], F16, tag="m", name=f"m{s}")
                    e2 = mepool.tile([128, 4096], F16, tag="e", name=f"e{s}")

                    for cc in range(2):
                        c = 2 * s + cc
                        b = c // 16
                        cp = c % 16
                        if cp == 0:
                            y_banks[b] = yps.tile([128, 512], F32, tag="ybank",
                                                  name=f"ybank{b}")
                        ybank = y_banks[b]
                        xT = xfT[:, 512 * cc:512 * (cc + 1)]

                        # --- L1: 4 row-tiled matmuls (K=32 strips)
                        zAB = [zps.tile([128, 1024], F32, tag="z", name=f"z{c}_{h}")
                               for h in range(2)]
                        for k in (range(4) if not ABLATE_L1 else []):
                            zsl = zAB[k // 2][:, 512 * (k % 2):512 * (k % 2) + 512]
                            row = slice(32 * k, 32 * k + 32)
                            nc.tensor.matmul(zsl, t_w1[row, 128 * c:128 * (c + 1)],
                                             xT[row, :], start=True, stop=True,
                                             tile_position=(32 * k, 0),
                                             skip_group_check=True)
                        # --- wlin matmuls (M=128, fp16 hi/lo pair)
                        if not ABLATE_L1:
                            nc.tensor.matmul(ybank[:, :], t_wlh[:, 128 * c:128 * (c + 1)],
                                             xT, start=(cp == 0), stop=False,
                                             skip_group_check=True)
                            nc.tensor.matmul(ybank[:, :], t_wlr[:, 128 * c:128 * (c + 1)],
                                             xT, start=False,
                                             stop=(ABLATE_EM and cp == 15),
                                             skip_group_check=True)
                        # --- m strips: DVE min-chain, or ACT relu-chain
                        for k in (range(4) if not ABLATE_ME else []):
                            zsl = zAB[k // 2][:, 512 * (k % 2):512 * (k % 2) + 512]
                            msl = m2[:, 2048 * cc + 512 * k:2048 * cc + 512 * k + 512]
                            bcol = t_b1[:, 4 * c + k:4 * c + k + 1]
                            if act_sc:
                                # q = relu(-(z + b1)); host stores b1c = -b1 here
                                nc.scalar.activation(msl, zsl, AF.Relu,
                                                     bias=bcol, scale=-1.0)
                            else:
                                nc.vector.tensor_scalar(msl, zsl, bcol,
                                                        0.0, AL.add, AL.min)
                    # --- e pass: one [128,4096] exp per superchunk
                    if not ABLATE_ME:
                        esc = -1.0 if act_sc else 1.0
                        nc.scalar.activation(e2[:], m2[:], AF.Exp, scale=esc)

                    def em_mms(s=s, m2=m2, e2=e2):
                        for cc in range(2):
                            c = 2 * s + cc
                            b = c // 16
                            ybk = y_banks[b]
                            last_chunk = (c % 16 == 15)
                            for k in (range(4) if not ABLATE_EM else []):
                                off = 2048 * cc + 512 * k
                                esl = e2[:, off:off + 512]
                                msl = m2[:, off:off + 512]
                                ysl = ybk[32 * k:32 * k + 32, :]
                                col = slice(128 * c + 32 * k, 128 * c + 32 * k + 32)
                                nc.tensor.matmul(
                                    ysl, t_w2e[:, col], esl,
                                    start=False, stop=False,
                                    tile_position=(0, 32 * k), skip_group_check=True)
                                nc.tensor.matmul(
                                    ysl, t_w2m[:, col], msl,
                                    start=False, stop=(last_chunk and k == 3),
                                    tile_position=(0, 32 * k), skip_group_check=True)
                            if last_chunk:
                                nc.vector.tensor_scalar(t_yfm[b][:], ybk[:],
                                                        t_c0[:, b:b + 1], None, AL.add)
                    nxt = em_mms
                else:
                    nxt = None
                old = pend.pop(0)
                if old is not None:
                    old()
                pend.append(nxt)

            # ---- norm + output (batch-major via permuted PE transpose)
            for j in range(4):
                yT = xin.tile([128, 512], F32, tag="xt", name=f"yT{j}")
                for b in range(BG):
                    pT = zps.tile([128, 128], F32, tag="z", name=f"pT{j}_{b}")
                    nc.tensor.transpose(pT[:], t_yfm[b][:, 128 * j:128 * (j + 1)],
                                        t_id[:])
                    nc.vector.tensor_copy(yT[:, 128 * b:128 * (b + 1)], pT[:])
                sq = xin.tile([128, 512], F32, tag="xt", name=f"sq{j}")
                nc.scalar.activation(sq[:], yT[:], AF.Square)
                ss = sml.tile([128, 1], F32, tag=f"ss{j}")
                nc.vector.reduce_sum(ss[:], sq[:], axis=mybir.AxisListType.X)
                s_ = sml.tile([128, 1], F32, tag=f"s{j}")
                nc.scalar.activation(s_[:], ss[:], AF.Sqrt)
                nc.vector.tensor_scalar(s_[:], s_[:], float(EPS), None, AL.max)
                r0 = sml.tile([128, 1], F32, tag=f"r0{j}")
                nc.vector.reciprocal(r0[:], s_[:])
                t1 = sml.tile([128, 1], F32, tag=f"t1{j}")
                nc.vector.tensor_tensor(t1[:], r0[:], r0[:], AL.mult)
                nc.vector.tensor_tensor(t1[:], t1[:], ss[:], AL.mult)
                nc.vector.tensor_scalar(t1[:], t1[:], -0.5, 1.5, AL.mult, AL.add)
                r1 = sml.tile([128, 1], F32, tag=f"r1{j}")
                nc.vector.tensor_tensor(r1[:], r0[:], t1[:], AL.mult)
                nc.vector.tensor_scalar(yT[:], yT[:], r1[:], None, AL.mult)
                nc.scalar.dma_start(y_out[128 * j:128 * (j + 1), :], yT[:])


def _pack_host(W1, b1, W2, b2):
    W1 = W1.astype(np.float32)
    b1 = b1.astype(np.float32)
    W2 = W2.astype(np.float32)
    b2 = b2.astype(np.float32)

    wlin = np.einsum('du,duv->dv', W2.astype(np.float64),
                     W1.astype(np.float64)).astype(np.float32)
    c0 = b2 + (W2 * b1).sum(-1) - W2.sum(-1)

    W1h = W1.astype(np.float16)
    wlh = wlin.astype(np.float16)
    wll = (wlin - wlh.astype(np.float32)).astype(np.float16)
    W2f = W2.astype(np.float16)

    w1hi = np.zeros((CH, 128, 128), np.float16)
    wlhi = np.zeros((CH, 128, 128), np.float16)
    wllo = np.zeros((CH, 128, 128), np.float16)
    w2e = np.zeros((CH, 128, 128), np.float16)
    b1c = np.zeros((CH, 128, 4), np.float32)
    c0s = np.zeros((128, BG), np.float32)

    for c in range(CH):
        cp = c % 16
        bi = c // 16
        for k in range(4):
            g0 = 8 * c + 2 * k
            g1 = g0 + 1
            w1hi[c, 32 * k:32 * k + 16, 0:64] = W1h[g0].T
            w1hi[c, 32 * k + 16:32 * k + 32, 64:128] = W1h[g1].T
            scol = 32 * k + 2 * cp
            wlhi[c, 32 * k:32 * k + 16, scol] = wlh[g0]
            wlhi[c, 32 * k + 16:32 * k + 32, scol + 1] = wlh[g1]
            wllo[c, 32 * k:32 * k + 16, scol] = wll[g0]
            wllo[c, 32 * k + 16:32 * k + 32, scol + 1] = wll[g1]
            w2e[c, 0:64, scol] = W2f[g0]
            w2e[c, 64:128, scol + 1] = W2f[g1]
            b1c[c, 0:64, k] = b1[g0]
            b1c[c, 64:128, k] = b1[g1]
            c0s[scol, bi] = c0[g0]
            c0s[scol + 1, bi] = c0[g1]
    w2m = -w2e
    for c in range(CH):
        if _act_chain(c):
            b1c[c] = -b1c[c]
            w2m[c] = -w2m[c]
    # permutation matrix: transpose output col j (= d-local) <- slot s
    ident = np.zeros((128, 128), dtype=np.float32)
    for cp in range(16):
        for k in range(4):
            for i_ in range(2):
                jcol = 8 * cp + 2 * k + i_
                slot = 32 * k + 2 * cp + i_
                ident[slot, jcol] = 1.0

    def flat(a):  # [CH,128,F] -> [128, CH*F]
        return np.ascontiguousarray(a.transpose(1, 0, 2).reshape(128, -1))

    return {"w1f": flat(w1hi), "wlh": flat(wlhi), "wlr": flat(wllo),
            "w2e": flat(w2e), "w2m": flat(w2m), "b1c": flat(b1c),
            "c0s": c0s, "ident": ident}


def kernel(x, W1, b1, W2, b2):
    x = np.ascontiguousarray(np.asarray(x, dtype=np.float32))
    packed = _pack_host(np.asarray(W1), np.asarray(b1),
                        np.asarray(W2), np.asarray(b2))
    if "nc" not in _cache:
        _cache["nc"] = _build()
    nc = _cache["nc"]
    in_maps = []
    for i in range(NCORE):
        m = dict(packed)
        m["x"] = x[i * R:(i + 1) * R]
        in_maps.append(m)
    res = bass_utils.run_bass_kernel_spmd(nc, in_maps, core_ids=list(range(NCORE)))
    out = np.concatenate([res.results[i]["y"] for i in range(NCORE)], axis=0)
    return out.astype(np.float32)
